# revision 7
# baseline (speedup 1.0000x reference)
"""Trainium2 Bass kernel for nn_ConvNormAct_38697655337417.

Computes, for x (16, 64, 128, 128) f32:
    z = cos(0.1) * cos(x)
    q = z + z^2 + z^3 + z^4            (elementwise "quantum conv")
    per-channel batchnorm (training stats over B,H,W), gamma/beta affine
    y = relu(norm) + x                 (residual)

Sharding: channel-parallel over 8 cores (8 channels/core). BN stats are
per-channel, so every core owns complete channels -> no collectives.
Per-core layout: [128 partitions = (c_local, b), 16384 free = H*W].

Per-core dataflow (tiles of F=2048, 8 tiles):
  DMA x -> SBUF (resident)
  DVE  add_range_wrap: t = wrap(x + pi/2) into [-pi, pi]
  ACT  Sin: u = cos(x)                      (in-place on t)
  ACT  Square(scale=c0): s = z^2
  DVE  stt: a = (u*c0) + s = z + z^2        (in-place on t)
  DVE  stt: q = (s+1)*a, accum -> sum(q)    (q resident)
  ACT  Square(q)+accum -> sum(q^2)          (dump to PSUM; some tiles on DVE)
  [stats fold: PE block-ones matmul -> per-channel mean/var -> A,B -> PE bcast]
  ACT  Relu(A*q + B) -> v
  GPSIMD v += x
  DMA  v -> out
"""
import math

import numpy as np

import concourse.bacc as bacc
import concourse.mybir as mybir
import concourse.tile as tile
from concourse.alu_op_type import AluOpType
from concourse.bass_utils import run_bass_kernel_spmd

B, C, H, W = 16, 64, 128, 128
NCORES = 8
CL = C // NCORES            # channels per core
P = CL * B                  # 128 partitions = (c_local, b)
FTOT = H * W                # 16384 free elements per partition
F = 2048                    # tile free size
NT = FTOT // F              # 8 tiles
N_STAT = B * H * W          # elements per channel for BN stats
INV_N = 1.0 / N_STAT        # 2^-18, exact
EPS = 1e-6
C0 = math.cos(0.1)
PI = math.pi
F32 = mybir.dt.float32

# tiles whose sum(q^2) is computed on DVE (tensor_tensor_reduce) instead of
# ACT (square+accum) -- load balance knob between the two engines
DVE_SUMSQ_TILES = set()  # ttr crashes TRN2 via this toolchain; keep all on ACT

_cached = None


def build_program(ftot=FTOT, f=F, dve_sumsq=None, debug_outs=False):
    FTOT_, F_ = ftot, f
    NT_ = FTOT_ // F_
    inv_n = 1.0 / (B * FTOT_)
    if dve_sumsq is None:
        dve_sumsq = DVE_SUMSQ_TILES
    nc = bacc.Bacc("TRN2", target_bir_lowering=False, debug=False)

    x_d = nc.dram_tensor("x", [P, FTOT_], F32, kind="ExternalInput").ap()
    gb_d = nc.dram_tensor("gb", [CL, 2], F32, kind="ExternalInput").ap()
    bo_d = nc.dram_tensor("bo", [P, CL], F32, kind="ExternalInput").ap()
    o8_d = nc.dram_tensor("o8", [CL, P], F32, kind="ExternalInput").ap()
    y_d = nc.dram_tensor("y", [P, FTOT_], F32, kind="ExternalOutput").ap()
    if debug_outs:
        dq_d = nc.dram_tensor("dq", [P, FTOT_], F32, kind="ExternalOutput").ap()
        dacc_d = nc.dram_tensor("dacc", [P, 3 * NT_], F32, kind="ExternalOutput").ap()
        drr_d = nc.dram_tensor("drr", [P, 2], F32, kind="ExternalOutput").ap()
        dab_d = nc.dram_tensor("dab", [P, 2], F32, kind="ExternalOutput").ap()

    AF = mybir.ActivationFunctionType

    with tile.TileContext(nc) as tc:
        with tc.tile_pool(name="xp", bufs=NT_) as xp, \
             tc.tile_pool(name="qp", bufs=NT_) as qp, \
             tc.tile_pool(name="tp", bufs=3) as tp, \
             tc.tile_pool(name="sp", bufs=2) as sp, \
             tc.tile_pool(name="vp", bufs=3) as vp, \
             tc.tile_pool(name="smp", bufs=1) as smp:

            gb = smp.tile([CL, 2], F32, tag="gb")
            nc.sync.dma_start(gb[:], gb_d[:])
            bo = smp.tile([P, CL], F32, tag="bo")
            nc.sync.dma_start(bo[:], bo_d[:])
            o8 = smp.tile([CL, P], F32, tag="o8")
            nc.sync.dma_start(o8[:], o8_d[:])

            acc1 = smp.tile([P, NT_], F32, tag="acc1")
            acc2a = smp.tile([P, NT_], F32, tag="acc2a")
            acc2b = smp.tile([P, NT_], F32, tag="acc2b")
            # acc2a/acc2b columns for tiles not written by that engine must
            # be zero for the final reduce
            nc.vector.memset(acc2a[:], 0.0)
            nc.vector.memset(acc2b[:], 0.0)

            xs, qs = [], []
            with tc.tile_pool(name="pdump", bufs=1, space="PSUM") as pdump:
                for i in range(NT_):
                    xt = xp.tile([P, F_], F32, tag="x")
                    nc.sync.dma_start(xt[:], x_d[:, bass_ts(i, F_)])
                    xs.append(xt)

                    t = tp.tile([P, F_], F32, tag="t")
                    nc.vector.add_range_wrap(t[:], xt[:], shift=PI / 2,
                                             bound=PI, period=2 * PI)
                    # u = sin(t) = cos(x), in-place
                    nc.scalar.activation(t[:], t[:], AF.Sin, bias=0.0,
                                         scale=1.0)
                    s = sp.tile([P, F_], F32, tag="s")
                    nc.scalar.activation(s[:], t[:], AF.Square, bias=0.0,
                                         scale=C0)
                    # a = (u * c0) + s, in-place on t
                    nc.vector.scalar_tensor_tensor(t[:], t[:], C0, s[:],
                                                   AluOpType.mult,
                                                   AluOpType.add)
                    q = qp.tile([P, F_], F32, tag="q")
                    nc.vector.scalar_tensor_tensor(q[:], s[:], 1.0, t[:],
                                                   AluOpType.add,
                                                   AluOpType.mult,
                                                   accum_out=acc1[:, i:i + 1])
                    qs.append(q)

                    if i in dve_sumsq:
                        dump = pdump.tile([P, F_], F32, tag="dumpv")
                        nc.vector.tensor_tensor_reduce(
                            dump[:], q[:], q[:], 1.0, 0.0,
                            AluOpType.mult, AluOpType.add,
                            accum_out=acc2b[:, i:i + 1])
                    else:
                        dump = pdump.tile([P, F_], F32, tag="dumpa")
                        nc.scalar.activation(dump[:], q[:], AF.Square,
                                             bias=0.0, scale=1.0,
                                             accum_out=acc2a[:, i:i + 1])

            # ---- stats fold ----
            rr = smp.tile([P, 2], F32, tag="rr")
            r2a = smp.tile([P, 1], F32, tag="r2a")
            nc.vector.reduce_sum(rr[:, 0:1], acc1[:], mybir.AxisListType.X)
            nc.vector.reduce_sum(rr[:, 1:2], acc2a[:], mybir.AxisListType.X)
            nc.vector.reduce_sum(r2a[:], acc2b[:], mybir.AxisListType.X)
            nc.vector.tensor_tensor(rr[:, 1:2], rr[:, 1:2], r2a[:],
                                    AluOpType.add)

            with tc.tile_pool(name="pstat", bufs=1, space="PSUM") as pstat:
                stp = pstat.tile([CL, 2], F32, tag="stp")
                nc.tensor.matmul(stp[:], bo[:], rr[:], start=True, stop=True)
                st = smp.tile([CL, 2], F32, tag="st")
                nc.vector.tensor_copy(st[:], stp[:])

                mean = smp.tile([CL, 1], F32, tag="mean")
                nc.vector.tensor_scalar_mul(mean[:], st[:, 0:1], inv_n)
                ex2p = smp.tile([CL, 1], F32, tag="ex2p")
                nc.vector.tensor_scalar(ex2p[:], st[:, 1:2], inv_n, EPS,
                                        AluOpType.mult, AluOpType.add)
                msq = smp.tile([CL, 1], F32, tag="msq")
                nc.vector.tensor_tensor(msq[:], mean[:], mean[:],
                                        AluOpType.mult)
                varep = smp.tile([CL, 1], F32, tag="varep")
                # varep = ex2p - msq = var + eps
                nc.vector.scalar_tensor_tensor(varep[:], msq[:], -1.0,
                                               ex2p[:], AluOpType.mult,
                                               AluOpType.add)
                sqv = smp.tile([CL, 1], F32, tag="sqv")
                nc.scalar.activation(sqv[:], varep[:], AF.Sqrt, bias=0.0,
                                     scale=1.0)
                rstd = smp.tile([CL, 1], F32, tag="rstd")
                nc.vector.reciprocal(rstd[:], sqv[:])

                AB8 = smp.tile([CL, 2], F32, tag="AB8")
                nc.vector.tensor_tensor(AB8[:, 0:1], gb[:, 0:1], rstd[:],
                                        AluOpType.mult)
                mA = smp.tile([CL, 1], F32, tag="mA")
                nc.vector.tensor_tensor(mA[:], mean[:], AB8[:, 0:1],
                                        AluOpType.mult)
                nc.vector.tensor_tensor(AB8[:, 1:2], gb[:, 1:2], mA[:],
                                        AluOpType.subtract)

                ABp = pstat.tile([P, 2], F32, tag="ABp")
                nc.tensor.matmul(ABp[:], o8[:], AB8[:], start=True, stop=True)
                ABs = smp.tile([P, 2], F32, tag="ABs")
                nc.vector.tensor_copy(ABs[:], ABp[:])

            if debug_outs:
                for i in range(NT_):
                    nc.sync.dma_start(dq_d[:, bass_ts(i, F_)], qs[i][:])
                nc.sync.dma_start(dacc_d[:, 0:NT_], acc1[:])
                nc.sync.dma_start(dacc_d[:, NT_:2 * NT_], acc2a[:])
                nc.sync.dma_start(dacc_d[:, 2 * NT_:3 * NT_], acc2b[:])
                nc.sync.dma_start(drr_d[:], rr[:])
                nc.sync.dma_start(dab_d[:], ABs[:])

            # ---- pass B: apply + residual + store ----
            for i in range(NT_):
                v = vp.tile([P, F_], F32, tag="v")
                nc.scalar.activation(v[:], qs[i][:], AF.Relu,
                                     bias=ABs[:, 1:2], scale=ABs[:, 0:1])
                nc.gpsimd.tensor_tensor(v[:], v[:], xs[i][:], AluOpType.add)
                nc.sync.dma_start(y_d[:, bass_ts(i, F_)], v[:])

    nc.compile()
    return nc


def bass_ts(i, size):
    import concourse.bass as bass
    return bass.ts(i, size)


def _shard_inputs(x, gamma, beta):
    arr = np.ascontiguousarray(x.transpose(1, 0, 2, 3)).reshape(C * B, H * W)
    bo = np.zeros((P, CL), dtype=np.float32)
    for k in range(P):
        bo[k, k // B] = 1.0
    o8 = np.zeros((CL, P), dtype=np.float32)
    for k in range(P):
        o8[k // B, k] = 1.0
    in_maps = []
    for c in range(NCORES):
        gb = np.stack([gamma[c * CL:(c + 1) * CL],
                       beta[c * CL:(c + 1) * CL]], axis=1)
        in_maps.append({
            "x": np.ascontiguousarray(arr[c * P:(c + 1) * P]),
            "gb": np.ascontiguousarray(gb.astype(np.float32)),
            "bo": bo,
            "o8": o8,
        })
    return in_maps


def kernel(x, gamma, beta):
    global _cached
    x = np.asarray(x, dtype=np.float32)
    gamma = np.asarray(gamma, dtype=np.float32)
    beta = np.asarray(beta, dtype=np.float32)
    if _cached is None:
        _cached = build_program()
    nc = _cached
    in_maps = _shard_inputs(x, gamma, beta)
    res = run_bass_kernel_spmd(nc, in_maps, core_ids=list(range(NCORES)))
    ys = np.concatenate([res.results[c]["y"] for c in range(NCORES)], axis=0)
    y = ys.reshape(C, B, H, W).transpose(1, 0, 2, 3)
    return np.ascontiguousarray(y)


if __name__ == "__main__":
    rng = np.random.default_rng(0)
    x = rng.standard_normal((B, C, H, W), dtype=np.float32)
    gamma = np.ones(C, dtype=np.float32)
    beta = np.zeros(C, dtype=np.float32)
    y = kernel(x, gamma, beta)
    print("out", y.shape, y.dtype)
